# revision 1
# baseline (speedup 1.0000x reference)
"""CenterLoss Trainium2 kernel.

loss = (sum_i clamp(||x_i - centers[labels_i]||^2, 1e-12, 1e12)
        + BS*(C_OUT-1)*1e-12) / BS

Masking the full [BS, C_OUT] distance matrix keeps exactly one distance
per row; the other BS*(C_OUT-1) clamped zeros are a host-side constant.

Data-parallel over batch across 8 NeuronCores, centers replicated.  Each
core gathers its 1024 label rows with dma_gather ucode instructions
(int16 idx limit => gather 512-byte center PAIRS at idx=label>>1, select
the even/odd half per row from the label parity on DVE).

The gather is split into PIECES so the pipeline overlaps: descriptor
generation of piece k+1 (Q7 ucode, ~8ns/row serial on the Pool engine)
hides the DMA transfer of piece k, and the DVE math of piece k hides
under gen/transfer of piece k+1.  Piece sizes shrink toward the end so
only a small transfer+math tail remains after the last descriptor gen.
The mlp ucode library swap (load_library) is issued at kernel start; its
~11us background load gates the first gather's ucode entry, so the idx
prep path (w16 DMA + DVE shift/cast, ready ~10.4us) is not critical.
Raw Bass blocks (no TileContext) avoid the Tile kernel-tail barrier.
Host sums partials in f64.

Host-side input prep is limited to sharding/replication and index-tensor
layout (int64->int32 and the 16-partition-wrapped x8-replicated index
layout the gather ucode requires); index arithmetic (>>1, &1) and all
data math run on device.
"""

import numpy as np

BS, C_OUT, D = 8192, 50000, 64
N_CORES = 8
ROWS = BS // N_CORES  # rows per core
P = 128  # SBUF partitions
RPP = ROWS // P  # rows per partition (row r lives at [r % P, r // P])
CLAMP_MIN, CLAMP_MAX = 1e-12, 1e12
PIECES = (512, 384, 128)  # gather split; multiples of 128 summing to ROWS

_CACHE = {}


def _build_program():
    import concourse.bacc as bacc
    import concourse.bass as bass
    import concourse.mybir as mybir
    from concourse.library_config import mlp

    nc = bacc.Bacc(
        "TRN2", target_bir_lowering=False, debug=False, num_devices=N_CORES
    )

    f32 = mybir.dt.float32
    i32 = mybir.dt.int32
    i16 = mybir.dt.int16

    x_d = nc.dram_tensor("x", [ROWS, D], f32, kind="ExternalInput")
    lab_d = nc.dram_tensor("labels", [ROWS], i32, kind="ExternalInput")
    w16_d = nc.dram_tensor("labels_w16", [P, ROWS // 16], i32, kind="ExternalInput")
    cen_d = nc.dram_tensor("centers", [C_OUT, D], f32, kind="ExternalInput")
    out_d = nc.dram_tensor("out", [P, RPP], f32, kind="ExternalOutput")

    from contextlib import ExitStack
    with ExitStack() as ctx:
        x_t = ctx.enter_context(nc.sbuf_tensor("x_t", [P, RPP * D], f32))
        w16_t = ctx.enter_context(nc.sbuf_tensor("w16_t", [P, ROWS // 16], i32))
        shr_t = ctx.enter_context(nc.sbuf_tensor("shr_t", [P, ROWS // 16], i32))
        idx_t = ctx.enter_context(nc.sbuf_tensor("idx_t", [P, ROWS // 16], i16))
        par_i = ctx.enter_context(nc.sbuf_tensor("par_i", [P, RPP], i32))
        parb_i = ctx.enter_context(nc.sbuf_tensor("parb_i", [P, RPP], i32))
        par_f = ctx.enter_context(nc.sbuf_tensor("par_f", [P, RPP], f32))
        pairs_t = ctx.enter_context(nc.sbuf_tensor("pairs_t", [P, RPP * 2 * D], f32))
        de_t = ctx.enter_context(nc.sbuf_tensor("de_t", [P, RPP * D], f32))
        do_t = ctx.enter_context(nc.sbuf_tensor("do_t", [P, RPP * D], f32))
        se_t = ctx.enter_context(nc.sbuf_tensor("se_t", [P, RPP], f32))
        so_t = ctx.enter_context(nc.sbuf_tensor("so_t", [P, RPP], f32))
        ds_t = ctx.enter_context(nc.sbuf_tensor("ds_t", [P, RPP], f32))
        s_t = ctx.enter_context(nc.sbuf_tensor("s_t", [P, RPP], f32))
        cl_t = ctx.enter_context(nc.sbuf_tensor("cl_t", [P, RPP], f32))
        s_w16 = ctx.enter_context(nc.semaphore("s_w16"))
        s_x = ctx.enter_context(nc.semaphore("s_x"))
        s_par = ctx.enter_context(nc.semaphore("s_par"))
        s_g = [ctx.enter_context(nc.semaphore(f"s_g{i}")) for i in range(len(PIECES))]
        s_dve = ctx.enter_context(nc.semaphore("s_dve"))
        s_v = ctx.enter_context(nc.semaphore("s_v"))
        s_out = ctx.enter_context(nc.semaphore("s_out"))
        block = ctx.enter_context(nc.Block())

        @block.sync
        def _(sync: bass.BassEngine):
            # w16 first: it gates the gather idx prep
            sync.dma_start(out=w16_t[:], in_=w16_d[:]).then_inc(s_w16, 16)
            # x rows: x_t[p, c*D:(c+1)*D] = x[c*128 + p] (gather row layout)
            sync.dma_start(
                out=x_t[:].rearrange("p (n m) -> p n m", m=D),
                in_=x_d[:].rearrange("(n p) m -> p n m", p=P),
            ).then_inc(s_x, 16)
            # parity source: par_i[p, c] = labels[c*128 + p]; element-strided
            # but fully shadowed by the library swap + gather
            with nc.allow_non_contiguous_dma(reason="4KB, hidden under gather"):
                sync.dma_start(
                    out=par_i[:], in_=lab_d[:].rearrange("(n p) -> p n", p=P)
                ).then_inc(s_par, 16)
            # writeback; no completion wait -- NEFF epilogue drains HWDGE
            sync.wait_ge(s_dve, 1)
            sync.dma_start(out=out_d[:], in_=cl_t[:]).then_inc(s_out, 16)

        @block.gpsimd
        def _(gpsimd: bass.BassGpSimd):
            # ucode swap starts here; ~11us background load gates the first
            # gather's ucode entry (the real critical path start)
            gpsimd.load_library(mlp)
            gpsimd.wait_ge(s_v, 2)  # idx_t ready (shift + cast on DVE)
            r0 = 0
            for q, n in enumerate(PIECES):
                gpsimd.dma_gather(
                    pairs_t[:].rearrange("p (n m) -> p n m", m=2 * D)[
                        :, r0 // 128 : (r0 + n) // 128, :
                    ],
                    cen_d[:].rearrange("(q t) m -> q (t m)", t=2),
                    idx_t[:, r0 // 16 : (r0 + n) // 16],
                    n,
                    n,
                    2 * D,
                ).then_inc(s_g[q], 16)
                r0 += n

        @block.vector
        def _(vector: bass.BassEngine):
            # DVE has no same-engine interlock: s_v counts completions
            nv = 0
            vector.wait_ge(s_w16, 16)
            vector.tensor_scalar(
                out=shr_t[:],
                in0=w16_t[:],
                scalar1=1,
                scalar2=None,
                op0=mybir.AluOpType.arith_shift_right,
            ).then_inc(s_v, 1)
            nv += 1
            vector.wait_ge(s_v, nv)
            vector.tensor_copy(out=idx_t[:], in_=shr_t[:]).then_inc(s_v, 1)
            nv += 1  # == 2: gather may start
            # parity -> f32; runs during the library swap window
            vector.wait_ge(s_par, 16)
            vector.tensor_scalar(
                out=parb_i[:],
                in0=par_i[:],
                scalar1=1,
                scalar2=None,
                op0=mybir.AluOpType.bitwise_and,
            ).then_inc(s_v, 1)
            nv += 1
            vector.wait_ge(s_v, nv)
            vector.tensor_copy(out=par_f[:], in_=parb_i[:]).then_inc(s_v, 1)
            nv += 1

            x_v = x_t[:].rearrange("p (n m) -> p n m", m=D)
            pairs_v = pairs_t[:].rearrange("p (n m) -> p n m", m=2 * D)
            de_v = de_t[:].rearrange("p (n m) -> p n m", m=D)
            do_v = do_t[:].rearrange("p (n m) -> p n m", m=D)

            vector.wait_ge(s_x, 16)
            r0 = 0
            for q, n in enumerate(PIECES):
                sl = slice(r0 // 128, (r0 + n) // 128)
                fl = slice(r0 // 128 * D, (r0 + n) // 128 * D)
                vector.wait_ge(s_g[q], 16)
                vector.tensor_tensor(
                    out=de_v[:, sl, :],
                    in0=x_v[:, sl, :],
                    in1=pairs_v[:, sl, 0:D],
                    op=mybir.AluOpType.subtract,
                ).then_inc(s_v, 1)
                nv += 1
                vector.tensor_tensor(
                    out=do_v[:, sl, :],
                    in0=x_v[:, sl, :],
                    in1=pairs_v[:, sl, D : 2 * D],
                    op=mybir.AluOpType.subtract,
                ).then_inc(s_v, 1)
                nv += 1
                vector.wait_ge(s_v, nv - 1)
                vector.tensor_tensor(
                    out=de_t[:, fl], in0=de_t[:, fl], in1=de_t[:, fl],
                    op=mybir.AluOpType.mult,
                ).then_inc(s_v, 1)
                nv += 1
                vector.wait_ge(s_v, nv - 1)
                vector.tensor_tensor(
                    out=do_t[:, fl], in0=do_t[:, fl], in1=do_t[:, fl],
                    op=mybir.AluOpType.mult,
                ).then_inc(s_v, 1)
                nv += 1
                vector.wait_ge(s_v, nv - 1)
                vector.reduce_sum(
                    out=se_t[:, sl], in_=de_v[:, sl, :], axis=mybir.AxisListType.X
                ).then_inc(s_v, 1)
                nv += 1
                vector.wait_ge(s_v, nv - 1)
                vector.reduce_sum(
                    out=so_t[:, sl], in_=do_v[:, sl, :], axis=mybir.AxisListType.X
                ).then_inc(s_v, 1)
                nv += 1
                r0 += n

            # s = s_even + par * (s_odd - s_even), then clamp
            vector.wait_ge(s_v, nv)
            vector.tensor_tensor(
                out=ds_t[:], in0=so_t[:], in1=se_t[:], op=mybir.AluOpType.subtract
            ).then_inc(s_v, 1)
            nv += 1
            vector.wait_ge(s_v, nv)
            vector.tensor_tensor(
                out=ds_t[:], in0=ds_t[:], in1=par_f[:], op=mybir.AluOpType.mult
            ).then_inc(s_v, 1)
            nv += 1
            vector.wait_ge(s_v, nv)
            vector.tensor_tensor(
                out=s_t[:], in0=se_t[:], in1=ds_t[:], op=mybir.AluOpType.add
            ).then_inc(s_v, 1)
            nv += 1
            vector.wait_ge(s_v, nv)
            vector.tensor_scalar(
                out=cl_t[:],
                in0=s_t[:],
                scalar1=CLAMP_MIN,
                scalar2=CLAMP_MAX,
                op0=mybir.AluOpType.max,
                op1=mybir.AluOpType.min,
            ).then_inc(s_dve, 1)

    nc.compile()
    return nc


def _get_program():
    if "nc" not in _CACHE:
        _CACHE["nc"] = _build_program()
    return _CACHE["nc"]


def _wrap16(labels_i32):
    # labels_w16[p, s] = labels[s*16 + p%16], replicated to 128 partitions
    base = labels_i32.reshape(ROWS // 16, 16).T  # [16, ROWS//16]
    return np.ascontiguousarray(np.tile(base, (P // 16, 1)))


def kernel(x, labels, centers, trace=False):
    from concourse.bass_utils import run_bass_kernel_spmd

    nc = _get_program()

    x = np.ascontiguousarray(np.asarray(x, dtype=np.float32))
    labels_i32 = np.ascontiguousarray(np.asarray(labels, dtype=np.int32))
    centers = np.ascontiguousarray(np.asarray(centers, dtype=np.float32))

    in_maps = []
    for i in range(N_CORES):
        lab_c = labels_i32[i * ROWS : (i + 1) * ROWS]
        in_maps.append(
            {
                "x": x[i * ROWS : (i + 1) * ROWS],
                "labels": lab_c,
                "labels_w16": _wrap16(lab_c),
                "centers": centers,
            }
        )

    res = run_bass_kernel_spmd(
        nc, in_maps, core_ids=list(range(N_CORES)), trace=trace
    )

    total = np.float64(0.0)
    for r in res.results:
        total += np.sum(r["out"], dtype=np.float64)
    # masked-out entries: BS*(C_OUT-1) zeros, each clamped to 1e-12
    total += np.float64(BS) * np.float64(C_OUT - 1) * 1e-12
    loss = np.float32(total / BS)

    if trace:
        _CACHE["last_exec_time_ns"] = res.exec_time_ns
        _CACHE["last_results"] = res
    return np.array(loss, dtype=np.float32)



# revision 9
# speedup vs baseline: 1.4663x; 1.4663x over previous
"""CenterLoss Trainium2 kernel.

loss = (sum_i clamp(||x_i - centers[labels_i]||^2, 1e-12, 1e12)
        + BS*(C_OUT-1)*1e-12) / BS

Masking the full [BS, C_OUT] distance matrix keeps exactly one distance
per row; the other BS*(C_OUT-1) clamped zeros are a host-side constant.

Data-parallel over batch across 8 NeuronCores, centers replicated.  Each
core gathers its 1024 label rows with indirect_dma_start (the resident
SWDGE indirect1d path, int32 indices), which -- unlike the mlp-library
dma_gather ucode -- needs no load_library, so the ~11us ucode IRAM load
disappears from the critical path and the gather can start as soon as
the 4KB label DMA lands (~2us into the kernel).

HW indirect1d semantics (measured): ONE index per destination
partition, gathering that partition's whole free extent contiguously
from src[idx[p]] -- the bass_interp multi-index-per-partition model
diverges from hardware.  So the 1024-row gather is RPP=8 calls, each
[128,1] indices -> [128, D] rows.  Each call gets its own completion
semaphore: per-engine sem increments make shared-sem thresholds racy.

Row r of the shard lives at SBUF [r // RPP, r % RPP]: partition-major
blocks make the x load a plain contiguous [128, 2KB] HWDGE copy and the
label load a [128, 32B] copy (both just reshape views on the host).
Descriptor generation of call k+1 overlaps the SDMA transfer of call k;
DVE consumes the gathered rows in two 4-column chunks (sub, square,
row-reduce), clamps, and the host sums the [128, RPP] partials in f64.

Host-side input prep is limited to sharding/replication, the int64 ->
int32 label cast, and reshape views; all data math runs on device.
Raw Bass blocks (no TileContext) avoid the Tile kernel-tail barrier.
"""

import numpy as np

BS, C_OUT, D = 8192, 50000, 64
N_CORES = 8
ROWS = BS // N_CORES  # rows per core
P = 128  # SBUF partitions
RPP = ROWS // P  # rows per partition (row r lives at [r // RPP, r % RPP])
CLAMP_MIN, CLAMP_MAX = 1e-12, 1e12
CHUNKS = (4, 4)  # DVE consumption chunks in columns; sums to RPP

_CACHE = {}


def _build_program():
    import concourse.bacc as bacc
    import concourse.bass as bass
    import concourse.mybir as mybir

    nc = bacc.Bacc(
        "TRN2", target_bir_lowering=False, debug=False, num_devices=N_CORES
    )

    f32 = mybir.dt.float32
    i32 = mybir.dt.int32

    x_d = nc.dram_tensor("x", [P, RPP * D], f32, kind="ExternalInput")
    lab_d = nc.dram_tensor("labels_pn", [P, RPP], i32, kind="ExternalInput")
    cen_d = nc.dram_tensor("centers", [C_OUT, D], f32, kind="ExternalInput")
    out_d = nc.dram_tensor("out", [P, RPP], f32, kind="ExternalOutput")

    from contextlib import ExitStack
    with ExitStack() as ctx:
        x_t = ctx.enter_context(nc.sbuf_tensor("x_t", [P, RPP * D], f32))
        idx_t = ctx.enter_context(nc.sbuf_tensor("idx_t", [P, RPP], i32))
        g_t = ctx.enter_context(nc.sbuf_tensor("g_t", [P, RPP * D], f32))
        d_t = ctx.enter_context(nc.sbuf_tensor("d_t", [P, RPP * D], f32))
        s_t = ctx.enter_context(nc.sbuf_tensor("s_t", [P, RPP], f32))
        cl_t = ctx.enter_context(nc.sbuf_tensor("cl_t", [P, RPP], f32))
        s_lab = ctx.enter_context(nc.semaphore("s_lab"))
        s_x = ctx.enter_context(nc.semaphore("s_x"))
        s_g = [ctx.enter_context(nc.semaphore(f"s_g{i}")) for i in range(RPP)]
        s_v = ctx.enter_context(nc.semaphore("s_v"))
        s_dve = ctx.enter_context(nc.semaphore("s_dve"))
        s_out = ctx.enter_context(nc.semaphore("s_out"))
        block = ctx.enter_context(nc.Block())

        @block.sync
        def _(sync: bass.BassEngine):
            # labels first: they gate the gather (critical path)
            sync.dma_start(out=idx_t[:], in_=lab_d[:]).then_inc(s_lab, 16)
            sync.dma_start(out=x_t[:], in_=x_d[:]).then_inc(s_x, 16)
            # writeback; no completion wait -- NEFF epilogue drains HWDGE
            sync.wait_ge(s_dve, 1)
            sync.dma_start(out=out_d[:], in_=cl_t[:]).then_inc(s_out, 16)

        @block.gpsimd
        def _(gpsimd: bass.BassGpSimd):
            gpsimd.wait_ge(s_lab, 16)
            for n in range(RPP):
                gpsimd.indirect_dma_start(
                    out=g_t[:, n * D : (n + 1) * D],
                    out_offset=None,
                    in_=cen_d[:],
                    in_offset=bass.IndirectOffsetOnAxis(
                        ap=idx_t[:, n : n + 1], axis=0
                    ),
                ).then_inc(s_g[n], 16)

        @block.vector
        def _(vector: bass.BassEngine):
            # DVE has no same-engine interlock: s_v counts completions
            nv = 0
            x_v = x_t[:].rearrange("p (n m) -> p n m", m=D)
            d_v = d_t[:].rearrange("p (n m) -> p n m", m=D)

            vector.wait_ge(s_x, 16)
            c0 = 0
            for ncols in CHUNKS:
                cs = slice(c0, c0 + ncols)
                fs = slice(c0 * D, (c0 + ncols) * D)
                for n in range(c0, c0 + ncols):
                    vector.wait_ge(s_g[n], 16)
                vector.tensor_tensor(
                    out=d_t[:, fs], in0=x_t[:, fs], in1=g_t[:, fs],
                    op=mybir.AluOpType.subtract,
                ).then_inc(s_v, 1)
                nv += 1
                vector.wait_ge(s_v, nv)
                vector.tensor_tensor(
                    out=d_t[:, fs], in0=d_t[:, fs], in1=d_t[:, fs],
                    op=mybir.AluOpType.mult,
                ).then_inc(s_v, 1)
                nv += 1
                vector.wait_ge(s_v, nv)
                vector.reduce_sum(
                    out=s_t[:, cs], in_=d_v[:, cs, :], axis=mybir.AxisListType.X
                ).then_inc(s_v, 1)
                nv += 1
                c0 += ncols

            vector.wait_ge(s_v, nv)
            vector.tensor_scalar(
                out=cl_t[:],
                in0=s_t[:],
                scalar1=CLAMP_MIN,
                scalar2=CLAMP_MAX,
                op0=mybir.AluOpType.max,
                op1=mybir.AluOpType.min,
            ).then_inc(s_dve, 1)

    nc.compile()
    return nc


def _get_program():
    if "nc" not in _CACHE:
        _CACHE["nc"] = _build_program()
    return _CACHE["nc"]


def kernel(x, labels, centers, trace=False):
    from concourse.bass_utils import run_bass_kernel_spmd

    nc = _get_program()

    x = np.ascontiguousarray(np.asarray(x, dtype=np.float32))
    labels_i32 = np.ascontiguousarray(np.asarray(labels, dtype=np.int32))
    centers = np.ascontiguousarray(np.asarray(centers, dtype=np.float32))

    in_maps = []
    for i in range(N_CORES):
        in_maps.append(
            {
                "x": x[i * ROWS : (i + 1) * ROWS].reshape(P, RPP * D),
                "labels_pn": labels_i32[i * ROWS : (i + 1) * ROWS].reshape(P, RPP),
                "centers": centers,
            }
        )

    res = run_bass_kernel_spmd(
        nc, in_maps, core_ids=list(range(N_CORES)), trace=trace
    )

    total = np.float64(0.0)
    for r in res.results:
        total += np.sum(r["out"], dtype=np.float64)
    # masked-out entries: BS*(C_OUT-1) zeros, each clamped to 1e-12
    total += np.float64(BS) * np.float64(C_OUT - 1) * 1e-12
    loss = np.float32(total / BS)

    if trace:
        _CACHE["last_exec_time_ns"] = res.exec_time_ns
        _CACHE["last_results"] = res
    return np.array(loss, dtype=np.float32)
